# revision 5
# baseline (speedup 1.0000x reference)
"""Trainium2 Bass kernel for nn_ConcatMesPassing (GNN message passing).

Reference computation:
    ne   = leaky_relu([u, e, v] @ W_e + b_e)                       # [E, 64]
    nbr  = leaky_relu([u, ne, v] @ W + b)                          # [E, 128]
    a    = leaky_relu(ne @ W_a + b_a)                              # [E]
    w    = softmax(node_edge_matrix * a + node_edge_mask, axis=1)  # [N, E]
    ctx  = elu(w @ nbr)                                            # [N, 128]
    returns (ctx, ne)

Key structural facts used here:
  * node_edge_matrix is binary and node_edge_mask == where(M>0, 0, -1e9),
    so the masked softmax row i is exactly  M_ij*exp(a_j) / sum_j M_ij*exp(a_j)
    (a is O(1) so exp needs no row-max subtraction).  The 256 MiB mask tensor
    never needs to be read.
  * ctx = elu( (M @ (s*nbr)) / (M @ s) )  with s = exp(a).

Sharding: edges are sharded 8 ways (2048 edges/core).  Each core runs the
edge MLPs for its slab and computes partial node aggregates
    ynum_k = (s*nbr)_k^T @ M[:, slab_k]^T   ([128, 4096], transposed)
    yden_k = s_k^T @ M[:, slab_k]^T         ([1, 4096])
over its 2048 columns of M.  The host sums the 8 partials, divides and
applies elu.  M column-slabs are pre-transposed/packed on the host so the
device streams them as fully-contiguous DMA loads and feeds the PE array
directly (contraction dim = edges on partitions).
"""

import os
import sys

import numpy as np

for _p in ("/opt/trn_rl_repo", "/root/.axon_site/_ro/trn_rl_repo"):
    if os.path.isdir(_p) and _p not in sys.path:
        sys.path.append(_p)

N_NODES = 4096
E_TOTAL = 16384
N_DIM = 128
E_DIM = 64
C_DIM = 128
N_CORES = 8
E_LOC = E_TOTAL // N_CORES          # 2048 edges per core
EB = E_LOC // 128                   # 16 edge blocks of 128
NCH = 8                             # node chunks per core
NCH_W = N_NODES // NCH              # 512 nodes per chunk

_CACHE = {}


def _build_program(use_lrelu=True):
    import concourse.bass as bass
    import concourse.mybir as mybir
    import concourse.tile as tile
    from concourse import bacc

    f32 = mybir.dt.float32
    f32r = mybir.dt.float32r
    AFT = mybir.ActivationFunctionType
    LRELU = AFT.Lrelu if use_lrelu else AFT.Relu  # Relu only for CoreSim checks

    nc = bacc.Bacc(None, target_bir_lowering=False, debug=False)

    # ---- DRAM I/O (per-core shards supplied by the host) ----
    uT_d = nc.dram_tensor("uT", [N_DIM, E_LOC], f32, kind="ExternalInput").ap()
    vT_d = nc.dram_tensor("vT", [N_DIM, E_LOC], f32, kind="ExternalInput").ap()
    eT_d = nc.dram_tensor("eT", [E_DIM, E_LOC], f32, kind="ExternalInput").ap()
    mt_d = nc.dram_tensor("mt", [NCH, 128, EB * NCH_W], f32r, kind="ExternalInput").ap()
    we_u_d = nc.dram_tensor("we_u", [128, E_DIM], f32, kind="ExternalInput").ap()
    we_e_d = nc.dram_tensor("we_e", [64, E_DIM], f32, kind="ExternalInput").ap()
    we_v_d = nc.dram_tensor("we_v", [128, E_DIM], f32, kind="ExternalInput").ap()
    w_u_d = nc.dram_tensor("w_u", [128, C_DIM], f32, kind="ExternalInput").ap()
    w_ne_d = nc.dram_tensor("w_ne", [64, C_DIM], f32, kind="ExternalInput").ap()
    w_v_d = nc.dram_tensor("w_v", [128, C_DIM], f32, kind="ExternalInput").ap()
    b_e_d = nc.dram_tensor("b_e", [E_DIM, 1], f32, kind="ExternalInput").ap()
    b_d = nc.dram_tensor("b", [C_DIM, 1], f32, kind="ExternalInput").ap()
    w_a_d = nc.dram_tensor("w_a", [E_DIM, 1], f32, kind="ExternalInput").ap()
    b_a_d = nc.dram_tensor("b_a", [128, 1], f32, kind="ExternalInput").ap()
    id_d = nc.dram_tensor("ident", [128, 128], f32, kind="ExternalInput").ap()

    neT_o = nc.dram_tensor("neT_out", [E_DIM, E_LOC], f32, kind="ExternalOutput").ap()
    ynum_o = nc.dram_tensor("ynum_out", [C_DIM, N_NODES], f32, kind="ExternalOutput").ap()
    yden_o = nc.dram_tensor("yden_out", [1, N_NODES], f32, kind="ExternalOutput").ap()

    def r(ap):
        return ap.bitcast(f32r)

    with tile.TileContext(nc) as tc:
        with (
            tc.tile_pool(name="const", bufs=1) as cpool,
            tc.tile_pool(name="feat", bufs=1) as fpool,
            tc.tile_pool(name="mtp", bufs=2) as mtpool,
            tc.tile_pool(name="outp", bufs=1) as opool,
            tc.tile_pool(name="pst", bufs=2, space="PSUM") as pst,
            tc.tile_pool(name="acc", bufs=2, space="PSUM") as acc,
        ):
            # constants
            we_u = cpool.tile([128, E_DIM], f32)
            we_e = cpool.tile([64, E_DIM], f32)
            we_v = cpool.tile([128, E_DIM], f32)
            w_u = cpool.tile([128, C_DIM], f32)
            w_ne = cpool.tile([64, C_DIM], f32)
            w_v = cpool.tile([128, C_DIM], f32)
            b_e = cpool.tile([E_DIM, 1], f32)
            b_c = cpool.tile([C_DIM, 1], f32)
            w_a = cpool.tile([E_DIM, 1], f32)
            b_a = cpool.tile([128, 1], f32)
            ident = cpool.tile([128, 128], f32)
            for sb, dr in [
                (we_u, we_u_d), (we_e, we_e_d), (we_v, we_v_d),
                (w_u, w_u_d), (w_ne, w_ne_d), (w_v, w_v_d),
                (b_e, b_e_d), (b_c, b_d), (w_a, w_a_d), (b_a, b_a_d),
                (ident, id_d),
            ]:
                nc.sync.dma_start(sb[:], dr)

            uT = fpool.tile([N_DIM, E_LOC], f32)
            vT = fpool.tile([N_DIM, E_LOC], f32)
            eT = fpool.tile([E_DIM, E_LOC], f32)
            nc.sync.dma_start(uT[:], uT_d)
            nc.sync.dma_start(vT[:], vT_d)
            nc.sync.dma_start(eT[:], eT_d)

            neT = fpool.tile([E_DIM, E_LOC], f32)
            nbrT = fpool.tile([C_DIM, E_LOC], f32)
            sN = fpool.tile([128, EB], f32)       # exp(a), edge-major, col per block
            gsc = fpool.tile([128, EB * C_DIM], f32r)  # s * nbr (rounded), edge-major
            sNr = fpool.tile([128, EB], f32r)     # rounded copy for the den matmul
            ynum_sb = opool.tile([C_DIM, N_NODES], f32)
            yden_sb = opool.tile([1, N_NODES], f32)

            # ---- MLP1: neT = leaky(W_e^T [u;e;v]) ----
            for c4 in range(4):
                sl = slice(c4 * 512, (c4 + 1) * 512)
                ps1 = pst.tile([E_DIM, 512], f32, tag="ps", name=f"ps1_{c4}")
                nc.tensor.matmul(ps1[:], we_u[:], uT[:, sl], start=True, stop=False)
                nc.tensor.matmul(ps1[:], we_e[:], eT[:, sl], start=False, stop=False)
                nc.tensor.matmul(ps1[:], we_v[:], vT[:, sl], start=False, stop=True)
                nc.scalar.activation(neT[:, sl], ps1[:], LRELU, bias=b_e[:], alpha=0.01)
            nc.sync.dma_start(neT_o, neT[:])

            # ---- MLP2: nbrT = leaky(W^T [u;ne;v]) ----
            for c4 in range(4):
                sl = slice(c4 * 512, (c4 + 1) * 512)
                ps2 = pst.tile([C_DIM, 512], f32, tag="ps", name=f"ps2_{c4}")
                nc.tensor.matmul(ps2[:], w_u[:], uT[:, sl], start=True, stop=False)
                nc.tensor.matmul(ps2[:], w_ne[:], neT[:, sl], start=False, stop=False)
                nc.tensor.matmul(ps2[:], w_v[:], vT[:, sl], start=False, stop=True)
                nc.scalar.activation(nbrT[:, sl], ps2[:], LRELU, bias=b_c[:], alpha=0.01)

            # ---- attention scalars, edge-major: s = exp(leaky(ne @ W_a + b_a)) ----
            for eb in range(EB):
                esl = slice(eb * 128, (eb + 1) * 128)
                pss = pst.tile([128, 1], f32, tag="ps", name=f"pss_{eb}")
                nc.tensor.matmul(pss[:], neT[:, esl], w_a[:], start=True, stop=True)
                lr = fpool.tile([128, 1], f32, tag="lr", name=f"lr_{eb}")
                nc.scalar.activation(lr[:], pss[:], LRELU, bias=b_a[:], alpha=0.01)
                nc.scalar.activation(sN[:, eb:eb + 1], lr[:], AFT.Exp)

            # ---- G = s * nbr in edge-major blocks (transpose nbrT via PE) ----
            for eb in range(EB):
                esl = slice(eb * 128, (eb + 1) * 128)
                pstr = pst.tile([128, C_DIM], f32, tag="ps", name=f"pstr_{eb}")
                nc.tensor.transpose(pstr[:], nbrT[:, esl], ident[:])
                nc.scalar.activation(
                    gsc[:, eb * C_DIM:(eb + 1) * C_DIM], pstr[:], AFT.Copy,
                    scale=sN[:, eb:eb + 1],
                )

            nc.vector.tensor_copy(sNr[:], sN[:])

            # ---- aggregation over M columns: ynum += G^T @ MT, yden += s^T @ MT ----
            for nch in range(NCH):
                mtc = mtpool.tile([128, EB * NCH_W], f32r, tag="mtc", name=f"mtc_{nch}")
                nc.sync.dma_start(mtc[:], mt_d[nch])
                pa = acc.tile([C_DIM, NCH_W], f32, tag="pa", name=f"pa_{nch}")
                pb = acc.tile([1, NCH_W], f32, tag="pb", name=f"pb_{nch}")
                for eb in range(EB):
                    rhs = mtc[:, eb * NCH_W:(eb + 1) * NCH_W]
                    nc.tensor.matmul(
                        pa[:], gsc[:, eb * C_DIM:(eb + 1) * C_DIM], rhs,
                        start=(eb == 0), stop=(eb == EB - 1),
                    )
                    nc.tensor.matmul(
                        pb[:], sNr[:, eb:eb + 1], rhs,
                        start=(eb == 0), stop=(eb == EB - 1),
                    )
                nsl = slice(nch * NCH_W, (nch + 1) * NCH_W)
                nc.scalar.activation(ynum_sb[:, nsl], pa[:], AFT.Copy)
                nc.vector.tensor_copy(yden_sb[:, nsl], pb[:])
            nc.sync.dma_start(ynum_o, ynum_sb[:])
            nc.sync.dma_start(yden_o, yden_sb[:])

    nc.compile()
    return nc


def _shard_inputs(inputs):
    """Build the 8 per-core input maps (host-side layout preparation)."""
    f32 = np.float32
    u = np.asarray(inputs["u_features"], dtype=f32)
    v = np.asarray(inputs["v_features"], dtype=f32)
    e = np.asarray(inputs["edge_features"], dtype=f32)
    M = np.asarray(inputs["node_edge_matrix"], dtype=f32)
    W_e = np.asarray(inputs["W_e"], dtype=f32)
    b_e = np.asarray(inputs["b_e"], dtype=f32)
    W = np.asarray(inputs["W"], dtype=f32)
    b = np.asarray(inputs["b"], dtype=f32)
    W_a = np.asarray(inputs["W_a"], dtype=f32)
    b_a = np.asarray(inputs["b_a"], dtype=f32)

    shared = {
        "we_u": np.ascontiguousarray(W_e[0:128]),
        "we_e": np.ascontiguousarray(W_e[128:192]),
        "we_v": np.ascontiguousarray(W_e[192:320]),
        "w_u": np.ascontiguousarray(W[0:128]),
        "w_ne": np.ascontiguousarray(W[128:192]),
        "w_v": np.ascontiguousarray(W[192:320]),
        "b_e": np.ascontiguousarray(b_e.reshape(E_DIM, 1)),
        "b": np.ascontiguousarray(b.reshape(C_DIM, 1)),
        "w_a": np.ascontiguousarray(W_a.reshape(E_DIM, 1)),
        "b_a": np.full((128, 1), float(b_a.reshape(-1)[0]), dtype=f32),
        "ident": np.eye(128, dtype=f32),
    }

    in_maps = []
    for k in range(N_CORES):
        sl = slice(k * E_LOC, (k + 1) * E_LOC)
        # M[:, sl].T -> [E_LOC, N]; pack as [NCH][128 part][EB*512] so each
        # node-chunk is one fully-contiguous 4 MiB DMA:
        #   mt[nch, p, eb*512 + j] = M[nch*512 + j, sl][eb*128 + p]
        mt = (
            M[:, sl]
            .T.reshape(EB, 128, NCH, NCH_W)
            .transpose(2, 1, 0, 3)
            .reshape(NCH, 128, EB * NCH_W)
        )
        in_map = {
            "uT": np.ascontiguousarray(u[sl].T),
            "vT": np.ascontiguousarray(v[sl].T),
            "eT": np.ascontiguousarray(e[sl].T),
            "mt": np.ascontiguousarray(mt),
        }
        in_map.update(shared)
        in_maps.append(in_map)
    return in_maps


def _postprocess(results, want_ne_fallback_inputs=None):
    ynum = np.zeros((C_DIM, N_NODES), np.float64)
    yden = np.zeros((1, N_NODES), np.float64)
    ne_slabs = []
    for res in results:
        ynum += res["ynum_out"]
        yden += res["yden_out"]
        ne_slabs.append(np.asarray(res["neT_out"]).T)
    new_edge = np.concatenate(ne_slabs, axis=0).astype(np.float32)
    ratio = (ynum / yden).T.astype(np.float32)  # [N, C]
    context = np.where(ratio > 0, ratio, np.expm1(ratio)).astype(np.float32)
    return context, new_edge


def kernel(**inputs):
    from concourse.bass_utils import run_bass_kernel_spmd

    if "nc" not in _CACHE:
        _CACHE["nc"] = _build_program()
    nc = _CACHE["nc"]

    in_maps = _shard_inputs(inputs)
    out = run_bass_kernel_spmd(nc, in_maps, core_ids=list(range(N_CORES)))
    context, new_edge = _postprocess(out.results)
    return context, new_edge


# revision 11
# speedup vs baseline: 1.3451x; 1.3451x over previous
"""Trainium2 Bass kernel for nn_ConcatMesPassing (GNN message passing).

Reference computation:
    ne   = leaky_relu([u, e, v] @ W_e + b_e)                       # [E, 64]
    nbr  = leaky_relu([u, ne, v] @ W + b)                          # [E, 128]
    a    = leaky_relu(ne @ W_a + b_a)                              # [E]
    w    = softmax(node_edge_matrix * a + node_edge_mask, axis=1)  # [N, E]
    ctx  = elu(w @ nbr)                                            # [N, 128]
    returns (ctx, ne)

Key structural facts used here:
  * node_edge_matrix is binary and node_edge_mask == where(M>0, 0, -1e9),
    so the masked softmax row i is exactly  M_ij*exp(a_j) / sum_j M_ij*exp(a_j)
    (a is O(1) so exp needs no row-max subtraction).  The 256 MiB mask tensor
    never needs to be read.
  * ctx = elu( (M @ (s*nbr)) / (M @ s) )  with s = exp(a).

Sharding: edges are sharded 8 ways (2048 edges/core).  Each core runs the
edge MLPs for its slab and computes partial node aggregates
    ynum_k = (s*nbr)_k^T @ M[:, slab_k]^T   ([128, 4096], transposed)
    yden_k = s_k^T @ M[:, slab_k]^T         ([1, 4096])
over its 2048 columns of M.  The host sums the 8 partials, divides and
applies elu.  M column-slabs are pre-transposed/packed on the host so the
device streams them as fully-contiguous DMA loads and feeds the PE array
directly (contraction dim = edges on partitions).
"""

import os
import sys

import numpy as np

for _p in ("/opt/trn_rl_repo", "/root/.axon_site/_ro/trn_rl_repo"):
    if os.path.isdir(_p) and _p not in sys.path:
        sys.path.append(_p)

N_NODES = 4096
E_TOTAL = 16384
N_DIM = 128
E_DIM = 64
C_DIM = 128
N_CORES = 8
E_LOC = E_TOTAL // N_CORES          # 2048 edges per core
EB = E_LOC // 128                   # 16 edge blocks of 128
NCH = 8                             # node chunks per core
NCH_W = N_NODES // NCH              # 512 nodes per chunk

_CACHE = {}


def _build_program(use_lrelu=True):
    import concourse.bass as bass
    import concourse.mybir as mybir
    import concourse.tile as tile
    from concourse import bacc

    f32 = mybir.dt.float32
    f32r = mybir.dt.float32r
    AFT = mybir.ActivationFunctionType
    LRELU = AFT.Lrelu if use_lrelu else AFT.Relu  # Relu only for CoreSim checks

    nc = bacc.Bacc(None, target_bir_lowering=False, debug=False)

    # ---- DRAM I/O (per-core shards supplied by the host) ----
    uT_d = nc.dram_tensor("uT", [N_DIM, E_LOC], f32r, kind="ExternalInput").ap()
    vT_d = nc.dram_tensor("vT", [N_DIM, E_LOC], f32r, kind="ExternalInput").ap()
    eT_d = nc.dram_tensor("eT", [E_DIM, E_LOC], f32r, kind="ExternalInput").ap()
    mt_d = nc.dram_tensor("mt", [NCH, 128, EB * NCH_W], f32r, kind="ExternalInput").ap()
    we_u_d = nc.dram_tensor("we_u", [128, E_DIM], f32r, kind="ExternalInput").ap()
    we_e_d = nc.dram_tensor("we_e", [64, E_DIM], f32r, kind="ExternalInput").ap()
    we_v_d = nc.dram_tensor("we_v", [128, E_DIM], f32r, kind="ExternalInput").ap()
    w_u_d = nc.dram_tensor("w_u", [128, C_DIM], f32r, kind="ExternalInput").ap()
    w_ne_d = nc.dram_tensor("w_ne", [64, C_DIM], f32r, kind="ExternalInput").ap()
    w_v_d = nc.dram_tensor("w_v", [128, C_DIM], f32r, kind="ExternalInput").ap()
    b_e_d = nc.dram_tensor("b_e", [E_DIM, 1], f32, kind="ExternalInput").ap()
    b_d = nc.dram_tensor("b", [C_DIM, 1], f32, kind="ExternalInput").ap()
    w_a_d = nc.dram_tensor("w_a", [E_DIM, 2], f32r, kind="ExternalInput").ap()  # fp32r needs even N
    b_a_d = nc.dram_tensor("b_a", [128, 1], f32, kind="ExternalInput").ap()
    id_d = nc.dram_tensor("ident", [128, 128], f32r, kind="ExternalInput").ap()

    neT_o = nc.dram_tensor("neT_out", [E_DIM, E_LOC], f32r, kind="ExternalOutput").ap()
    ynum_o = nc.dram_tensor("ynum_out", [C_DIM, N_NODES], f32, kind="ExternalOutput").ap()
    yden_o = nc.dram_tensor("yden_out", [1, N_NODES], f32, kind="ExternalOutput").ap()

    def r(ap):
        return ap.bitcast(f32r)

    with tile.TileContext(nc) as tc:
        with (
            tc.tile_pool(name="const", bufs=1) as cpool,
            tc.tile_pool(name="feat", bufs=1) as fpool,
            tc.tile_pool(name="mtp", bufs=6) as mtpool,
            tc.tile_pool(name="outp", bufs=1) as opool,
            tc.tile_pool(name="pst", bufs=2, space="PSUM") as pst,
            tc.tile_pool(name="acc", bufs=2, space="PSUM") as acc,
        ):
            # constants
            we_u = cpool.tile([128, E_DIM], f32r)
            we_e = cpool.tile([64, E_DIM], f32r)
            we_v = cpool.tile([128, E_DIM], f32r)
            w_u = cpool.tile([128, C_DIM], f32r)
            w_ne = cpool.tile([64, C_DIM], f32r)
            w_v = cpool.tile([128, C_DIM], f32r)
            b_e = cpool.tile([E_DIM, 1], f32)
            b_c = cpool.tile([C_DIM, 1], f32)
            w_a = cpool.tile([E_DIM, 2], f32r)
            b_a = cpool.tile([128, 1], f32)
            ident = cpool.tile([128, 128], f32r)
            for sb, dr in [
                (we_u, we_u_d), (we_e, we_e_d), (we_v, we_v_d),
                (w_u, w_u_d), (w_ne, w_ne_d), (w_v, w_v_d),
                (b_e, b_e_d), (b_c, b_d), (w_a, w_a_d), (b_a, b_a_d),
                (ident, id_d),
            ]:
                nc.sync.dma_start(sb[:], dr)

            uT = fpool.tile([N_DIM, E_LOC], f32r)
            vT = fpool.tile([N_DIM, E_LOC], f32r)
            eT = fpool.tile([E_DIM, E_LOC], f32r)
            nc.sync.dma_start(uT[:], uT_d)
            nc.sync.dma_start(vT[:], vT_d)
            nc.sync.dma_start(eT[:], eT_d)

            neT = fpool.tile([E_DIM, E_LOC], f32r)
            nbrT = fpool.tile([C_DIM, E_LOC], f32r)
            sN = fpool.tile([128, 2 * EB], f32)   # exp(a), edge-major, col 2*eb per block
            gsc = fpool.tile([128, EB * C_DIM], f32r)  # s * nbr (rounded), edge-major
            sNr = fpool.tile([128, 2 * EB], f32r)  # rounded copy for the den matmul
            ynum_sb = opool.tile([C_DIM, N_NODES], f32)
            yden_sb = opool.tile([1, N_NODES], f32)

            # ---- MLP1: neT = leaky(W_e^T [u;e;v]) ----
            for c4 in range(4):
                sl = slice(c4 * 512, (c4 + 1) * 512)
                ps1 = pst.tile([E_DIM, 512], f32, tag="ps", name=f"ps1_{c4}")
                nc.tensor.matmul(ps1[:], we_u[:], uT[:, sl], start=True, stop=False)
                nc.tensor.matmul(ps1[:], we_e[:], eT[:, sl], start=False, stop=False)
                nc.tensor.matmul(ps1[:], we_v[:], vT[:, sl], start=False, stop=True)
                nc.scalar.activation(neT[:, sl], ps1[:], LRELU, bias=b_e[:], alpha=0.01)
            nc.sync.dma_start(neT_o, neT[:])

            # ---- MLP2: nbrT = leaky(W^T [u;ne;v]) ----
            for c4 in range(4):
                sl = slice(c4 * 512, (c4 + 1) * 512)
                ps2 = pst.tile([C_DIM, 512], f32, tag="ps", name=f"ps2_{c4}")
                nc.tensor.matmul(ps2[:], w_u[:], uT[:, sl], start=True, stop=False)
                nc.tensor.matmul(ps2[:], w_ne[:], neT[:, sl], start=False, stop=False)
                nc.tensor.matmul(ps2[:], w_v[:], vT[:, sl], start=False, stop=True)
                nc.scalar.activation(nbrT[:, sl], ps2[:], LRELU, bias=b_c[:], alpha=0.01)

            # ---- attention scalars, edge-major: s = exp(leaky(ne @ W_a + b_a)) ----
            # All 16 per-block matmuls land in one [128, EB] psum tile so the
            # Lrelu and Exp each run once (ACT table reloads on every function
            # switch, ~1.3us each -- batching cut 32 reloads to 2).
            pss = pst.tile([128, 2 * EB], f32, tag="ps", name="pss")
            for eb in range(EB):
                esl = slice(eb * 128, (eb + 1) * 128)
                nc.tensor.matmul(pss[:, 2 * eb:2 * eb + 2], neT[:, esl], w_a[:],
                                 start=True, stop=True)
            lr = fpool.tile([128, 2 * EB], f32, name="lr")
            nc.scalar.activation(lr[:], pss[:], LRELU, bias=b_a[:], alpha=0.01)
            nc.scalar.activation(sN[:], lr[:], AFT.Exp)

            # ---- G = s * nbr in edge-major blocks (transpose nbrT via PE) ----
            for eb in range(EB):
                esl = slice(eb * 128, (eb + 1) * 128)
                pstr = pst.tile([128, C_DIM], f32r, tag="ps", name=f"pstr_{eb}")
                nc.tensor.transpose(pstr[:], nbrT[:, esl], ident[:])
                nc.scalar.activation(
                    gsc[:, eb * C_DIM:(eb + 1) * C_DIM], pstr[:], AFT.Copy,
                    scale=sN[:, 2 * eb:2 * eb + 1],
                )

            nc.vector.tensor_copy(sNr[:], sN[:])

            # ---- aggregation over M columns: ynum += G^T @ MT, yden += s^T @ MT ----
            # Each node-chunk's 4 MiB of M^T streams in as two 2 MiB halves
            # (finer prefetch grain; bufs=6 keeps DMA ~12 MiB ahead so it never
            # stalls behind the PE).  Outputs drain per-chunk to overlap the
            # store with the remaining aggregation.
            HEB = EB // 2  # 8 edge blocks per half
            for nch in range(NCH):
                pa = acc.tile([C_DIM, NCH_W], f32, tag="pa", name=f"pa_{nch}")
                pb = acc.tile([1, NCH_W], f32, tag="pb", name=f"pb_{nch}")
                for h in range(2):
                    mtc = mtpool.tile([128, HEB * NCH_W], f32r, tag="mtc",
                                      name=f"mtc_{nch}_{h}")
                    nc.sync.dma_start(
                        mtc[:], mt_d[nch][:, h * HEB * NCH_W:(h + 1) * HEB * NCH_W]
                    )
                    for k in range(HEB):
                        eb = h * HEB + k
                        rhs = mtc[:, k * NCH_W:(k + 1) * NCH_W]
                        nc.tensor.matmul(
                            pa[:], gsc[:, eb * C_DIM:(eb + 1) * C_DIM], rhs,
                            start=(eb == 0), stop=(eb == EB - 1),
                        )
                        nc.tensor.matmul(
                            pb[:], sNr[:, 2 * eb:2 * eb + 1], rhs,
                            start=(eb == 0), stop=(eb == EB - 1),
                        )
                nsl = slice(nch * NCH_W, (nch + 1) * NCH_W)
                nc.scalar.activation(ynum_sb[:, nsl], pa[:], AFT.Copy)
                nc.vector.tensor_copy(yden_sb[:, nsl], pb[:])
                nc.sync.dma_start(ynum_o[:, nsl], ynum_sb[:, nsl])
                nc.sync.dma_start(yden_o[:, nsl], yden_sb[:, nsl])

    nc.compile()
    return nc


def _shard_inputs(inputs):
    """Build the 8 per-core input maps (host-side layout preparation)."""
    f32 = np.float32
    u = np.asarray(inputs["u_features"], dtype=f32)
    v = np.asarray(inputs["v_features"], dtype=f32)
    e = np.asarray(inputs["edge_features"], dtype=f32)
    M = np.asarray(inputs["node_edge_matrix"], dtype=f32)
    W_e = np.asarray(inputs["W_e"], dtype=f32)
    b_e = np.asarray(inputs["b_e"], dtype=f32)
    W = np.asarray(inputs["W"], dtype=f32)
    b = np.asarray(inputs["b"], dtype=f32)
    W_a = np.asarray(inputs["W_a"], dtype=f32)
    b_a = np.asarray(inputs["b_a"], dtype=f32)

    shared = {
        "we_u": np.ascontiguousarray(W_e[0:128]),
        "we_e": np.ascontiguousarray(W_e[128:192]),
        "we_v": np.ascontiguousarray(W_e[192:320]),
        "w_u": np.ascontiguousarray(W[0:128]),
        "w_ne": np.ascontiguousarray(W[128:192]),
        "w_v": np.ascontiguousarray(W[192:320]),
        "b_e": np.ascontiguousarray(b_e.reshape(E_DIM, 1)),
        "b": np.ascontiguousarray(b.reshape(C_DIM, 1)),
        "w_a": np.ascontiguousarray(np.repeat(W_a.reshape(E_DIM, 1), 2, axis=1)),
        "b_a": np.full((128, 1), float(b_a.reshape(-1)[0]), dtype=f32),
        "ident": np.eye(128, dtype=f32),
    }

    in_maps = []
    for k in range(N_CORES):
        sl = slice(k * E_LOC, (k + 1) * E_LOC)
        # M[:, sl].T -> [E_LOC, N]; pack as [NCH][128 part][EB*512] so each
        # node-chunk is one fully-contiguous 4 MiB DMA:
        #   mt[nch, p, eb*512 + j] = M[nch*512 + j, sl][eb*128 + p]
        mt = (
            M[:, sl]
            .T.reshape(EB, 128, NCH, NCH_W)
            .transpose(2, 1, 0, 3)
            .reshape(NCH, 128, EB * NCH_W)
        )
        in_map = {
            "uT": np.ascontiguousarray(u[sl].T),
            "vT": np.ascontiguousarray(v[sl].T),
            "eT": np.ascontiguousarray(e[sl].T),
            "mt": np.ascontiguousarray(mt),
        }
        in_map.update(shared)
        in_maps.append(in_map)
    return in_maps


def _postprocess(results, want_ne_fallback_inputs=None):
    ynum = np.zeros((C_DIM, N_NODES), np.float64)
    yden = np.zeros((1, N_NODES), np.float64)
    ne_slabs = []
    for res in results:
        ynum += res["ynum_out"]
        yden += res["yden_out"]
        ne_slabs.append(np.asarray(res["neT_out"]).T)
    new_edge = np.concatenate(ne_slabs, axis=0).astype(np.float32)
    ratio = (ynum / yden).T.astype(np.float32)  # [N, C]
    context = np.where(ratio > 0, ratio, np.expm1(ratio)).astype(np.float32)
    return context, new_edge


def kernel(**inputs):
    from concourse.bass_utils import run_bass_kernel_spmd

    if "nc" not in _CACHE:
        _CACHE["nc"] = _build_program()
    nc = _CACHE["nc"]

    in_maps = _shard_inputs(inputs)
    out = run_bass_kernel_spmd(nc, in_maps, core_ids=list(range(N_CORES)))
    context, new_edge = _postprocess(out.results)
    return context, new_edge


# revision 12
# speedup vs baseline: 1.4869x; 1.1054x over previous
"""Trainium2 Bass kernel for nn_ConcatMesPassing (GNN message passing).

Reference computation:
    ne   = leaky_relu([u, e, v] @ W_e + b_e)                       # [E, 64]
    nbr  = leaky_relu([u, ne, v] @ W + b)                          # [E, 128]
    a    = leaky_relu(ne @ W_a + b_a)                              # [E]
    w    = softmax(node_edge_matrix * a + node_edge_mask, axis=1)  # [N, E]
    ctx  = elu(w @ nbr)                                            # [N, 128]
    returns (ctx, ne)

Key structural facts used here:
  * node_edge_matrix is binary and node_edge_mask == where(M>0, 0, -1e9),
    so the masked softmax row i is exactly  M_ij*exp(a_j) / sum_j M_ij*exp(a_j)
    (a is O(1) so exp needs no row-max subtraction).  The 256 MiB mask tensor
    never needs to be read.
  * ctx = elu( (M @ (s*nbr)) / (M @ s) )  with s = exp(a).

Sharding: edges are sharded 8 ways (2048 edges/core).  Each core runs the
edge MLPs for its slab and computes partial node aggregates
    ynum_k = (s*nbr)_k^T @ M[:, slab_k]^T   ([128, 4096], transposed)
    yden_k = s_k^T @ M[:, slab_k]^T         ([1, 4096])
over its 2048 columns of M.  The host sums the 8 partials, divides and
applies elu.  M column-slabs are pre-transposed/packed on the host so the
device streams them as fully-contiguous DMA loads and feeds the PE array
directly (contraction dim = edges on partitions).
"""

import os
import sys

import numpy as np

for _p in ("/opt/trn_rl_repo", "/root/.axon_site/_ro/trn_rl_repo"):
    if os.path.isdir(_p) and _p not in sys.path:
        sys.path.append(_p)

N_NODES = 4096
E_TOTAL = 16384
N_DIM = 128
E_DIM = 64
C_DIM = 128
N_CORES = 8
E_LOC = E_TOTAL // N_CORES          # 2048 edges per core
EB = E_LOC // 128                   # 16 edge blocks of 128
NCH = 8                             # node chunks per core
NCH_W = N_NODES // NCH              # 512 nodes per chunk

_CACHE = {}


def _build_program(use_lrelu=True):
    import concourse.bass as bass
    import concourse.mybir as mybir
    import concourse.tile as tile
    from concourse import bacc

    f32 = mybir.dt.float32
    f32r = mybir.dt.float32r
    AFT = mybir.ActivationFunctionType
    LRELU = AFT.Lrelu if use_lrelu else AFT.Relu  # Relu only for CoreSim checks

    nc = bacc.Bacc(None, target_bir_lowering=False, debug=False)

    # ---- DRAM I/O (per-core shards supplied by the host) ----
    uT_d = nc.dram_tensor("uT", [N_DIM, E_LOC], f32r, kind="ExternalInput").ap()
    vT_d = nc.dram_tensor("vT", [N_DIM, E_LOC], f32r, kind="ExternalInput").ap()
    eT_d = nc.dram_tensor("eT", [E_DIM, E_LOC], f32r, kind="ExternalInput").ap()
    mt_d = nc.dram_tensor("mt", [NCH, 128, EB * NCH_W], f32r, kind="ExternalInput").ap()
    we_u_d = nc.dram_tensor("we_u", [128, E_DIM], f32r, kind="ExternalInput").ap()
    we_e_d = nc.dram_tensor("we_e", [64, E_DIM], f32r, kind="ExternalInput").ap()
    we_v_d = nc.dram_tensor("we_v", [128, E_DIM], f32r, kind="ExternalInput").ap()
    w_u_d = nc.dram_tensor("w_u", [128, C_DIM], f32r, kind="ExternalInput").ap()
    w_ne_d = nc.dram_tensor("w_ne", [64, C_DIM], f32r, kind="ExternalInput").ap()
    w_v_d = nc.dram_tensor("w_v", [128, C_DIM], f32r, kind="ExternalInput").ap()
    b_e_d = nc.dram_tensor("b_e", [E_DIM, 1], f32, kind="ExternalInput").ap()
    b_d = nc.dram_tensor("b", [C_DIM, 1], f32, kind="ExternalInput").ap()
    w_a_d = nc.dram_tensor("w_a", [E_DIM, 2], f32r, kind="ExternalInput").ap()  # fp32r needs even N
    b_a_d = nc.dram_tensor("b_a", [128, 1], f32, kind="ExternalInput").ap()
    id_d = nc.dram_tensor("ident", [128, 128], f32r, kind="ExternalInput").ap()

    neT_o = nc.dram_tensor("neT_out", [E_DIM, E_LOC], f32r, kind="ExternalOutput").ap()
    ynum_o = nc.dram_tensor("ynum_out", [C_DIM, N_NODES], f32, kind="ExternalOutput").ap()
    yden_o = nc.dram_tensor("yden_out", [1, N_NODES], f32, kind="ExternalOutput").ap()

    def r(ap):
        return ap.bitcast(f32r)

    with tile.TileContext(nc) as tc:
        with (
            tc.tile_pool(name="const", bufs=1) as cpool,
            tc.tile_pool(name="feat", bufs=1) as fpool,
            tc.tile_pool(name="mtp", bufs=6) as mtpool,
            tc.tile_pool(name="outp", bufs=1) as opool,
            tc.tile_pool(name="pst", bufs=2, space="PSUM") as pst,
            tc.tile_pool(name="acc", bufs=3, space="PSUM") as acc,
        ):
            # constants
            we_u = cpool.tile([128, E_DIM], f32r)
            we_e = cpool.tile([64, E_DIM], f32r)
            we_v = cpool.tile([128, E_DIM], f32r)
            w_u = cpool.tile([128, C_DIM], f32r)
            w_ne = cpool.tile([64, C_DIM], f32r)
            w_v = cpool.tile([128, C_DIM], f32r)
            b_e = cpool.tile([E_DIM, 1], f32)
            b_c = cpool.tile([C_DIM, 1], f32)
            w_a = cpool.tile([E_DIM, 2], f32r)
            b_a = cpool.tile([128, 1], f32)
            ident = cpool.tile([128, 128], f32r)
            for sb, dr in [
                (we_u, we_u_d), (we_e, we_e_d), (we_v, we_v_d),
                (w_u, w_u_d), (w_ne, w_ne_d), (w_v, w_v_d),
                (b_e, b_e_d), (b_c, b_d), (w_a, w_a_d), (b_a, b_a_d),
                (ident, id_d),
            ]:
                nc.sync.dma_start(sb[:], dr)

            uT = fpool.tile([N_DIM, E_LOC], f32r)
            vT = fpool.tile([N_DIM, E_LOC], f32r)
            eT = fpool.tile([E_DIM, E_LOC], f32r)
            nc.sync.dma_start(uT[:], uT_d)
            nc.sync.dma_start(vT[:], vT_d)
            nc.sync.dma_start(eT[:], eT_d)

            neT = fpool.tile([E_DIM, E_LOC], f32r)
            nbrT = fpool.tile([C_DIM, E_LOC], f32r)
            sN = fpool.tile([128, 2 * EB], f32)   # exp(a), edge-major, col 2*eb per block
            gsc = fpool.tile([128, EB * C_DIM], f32r)  # s * nbr (rounded), edge-major
            sNr = fpool.tile([128, 2 * EB], f32r)  # rounded copy for the den matmul
            ynum_sb = opool.tile([C_DIM, N_NODES], f32)
            yden_sb = opool.tile([1, N_NODES], f32)

            # ---- MLP1: neT = leaky(W_e^T [u;e;v]) ----
            for c4 in range(4):
                sl = slice(c4 * 512, (c4 + 1) * 512)
                ps1 = pst.tile([E_DIM, 512], f32, tag="ps", name=f"ps1_{c4}")
                nc.tensor.matmul(ps1[:], we_u[:], uT[:, sl], start=True, stop=False)
                nc.tensor.matmul(ps1[:], we_e[:], eT[:, sl], start=False, stop=False)
                nc.tensor.matmul(ps1[:], we_v[:], vT[:, sl], start=False, stop=True)
                nc.scalar.activation(neT[:, sl], ps1[:], LRELU, bias=b_e[:], alpha=0.01)
            nc.sync.dma_start(neT_o, neT[:])

            # ---- MLP2: nbrT = leaky(W^T [u;ne;v]) ----
            for c4 in range(4):
                sl = slice(c4 * 512, (c4 + 1) * 512)
                ps2 = pst.tile([C_DIM, 512], f32, tag="ps", name=f"ps2_{c4}")
                nc.tensor.matmul(ps2[:], w_u[:], uT[:, sl], start=True, stop=False)
                nc.tensor.matmul(ps2[:], w_ne[:], neT[:, sl], start=False, stop=False)
                nc.tensor.matmul(ps2[:], w_v[:], vT[:, sl], start=False, stop=True)
                nc.scalar.activation(nbrT[:, sl], ps2[:], LRELU, bias=b_c[:], alpha=0.01)

            # ---- attention scalars, edge-major: s = exp(leaky(ne @ W_a + b_a)) ----
            # All 16 per-block matmuls land in one [128, EB] psum tile so the
            # Lrelu and Exp each run once (ACT table reloads on every function
            # switch, ~1.3us each -- batching cut 32 reloads to 2).
            pss = pst.tile([128, 2 * EB], f32, tag="ps", name="pss")
            for eb in range(EB):
                esl = slice(eb * 128, (eb + 1) * 128)
                nc.tensor.matmul(pss[:, 2 * eb:2 * eb + 2], neT[:, esl], w_a[:],
                                 start=True, stop=True)
            lr = fpool.tile([128, 2 * EB], f32, name="lr")
            nc.scalar.activation(lr[:], pss[:], LRELU, bias=b_a[:], alpha=0.01)
            nc.scalar.activation(sN[:], lr[:], AFT.Exp)

            # ---- G = s * nbr in edge-major blocks (transpose nbrT via PE) ----
            for eb in range(EB):
                esl = slice(eb * 128, (eb + 1) * 128)
                pstr = pst.tile([128, C_DIM], f32r, tag="ps", name=f"pstr_{eb}")
                nc.tensor.transpose(pstr[:], nbrT[:, esl], ident[:])
                nc.scalar.activation(
                    gsc[:, eb * C_DIM:(eb + 1) * C_DIM], pstr[:], AFT.Copy,
                    scale=sN[:, 2 * eb:2 * eb + 1],
                )

            nc.vector.tensor_copy(sNr[:], sN[:])

            # ---- aggregation over M columns: ynum += G^T @ MT, yden += s^T @ MT ----
            # Node chunks are processed in pairs that share each gsc[eb]/sN[eb]
            # stationary load (halves LDWEIGHTS) and keep the PE dense.  M^T
            # streams as 2 MiB halves on the scalar-engine HWDGE ring so the
            # feature loads on the sync ring aren't stuck behind the prefetch
            # queue.  Outputs drain per-chunk to overlap stores with compute.
            HEB = EB // 2  # 8 edge blocks per half-chunk
            for pr in range(NCH // 2):
                pas = [acc.tile([C_DIM, NCH_W], f32, tag="pa", name=f"pa_{pr}_{j}")
                       for j in range(2)]
                pbs = [acc.tile([1, NCH_W], f32, tag="pb", name=f"pb_{pr}_{j}")
                       for j in range(2)]
                for h in range(2):
                    mts = []
                    for j in range(2):
                        nch = 2 * pr + j
                        mtc = mtpool.tile([128, HEB * NCH_W], f32r, tag="mtc",
                                          name=f"mtc_{nch}_{h}")
                        nc.scalar.dma_start(
                            mtc[:],
                            mt_d[nch][:, h * HEB * NCH_W:(h + 1) * HEB * NCH_W],
                        )
                        mts.append(mtc)
                    for k in range(HEB):
                        eb = h * HEB + k
                        st, sp = (eb == 0), (eb == EB - 1)
                        g = gsc[:, eb * C_DIM:(eb + 1) * C_DIM]
                        sv = sNr[:, 2 * eb:2 * eb + 1]
                        for j in range(2):
                            rhs = mts[j][:, k * NCH_W:(k + 1) * NCH_W]
                            nc.tensor.matmul(pas[j][:], g, rhs, start=st, stop=sp)
                        for j in range(2):
                            rhs = mts[j][:, k * NCH_W:(k + 1) * NCH_W]
                            nc.tensor.matmul(pbs[j][:], sv, rhs, start=st, stop=sp)
                for j in range(2):
                    nch = 2 * pr + j
                    nsl = slice(nch * NCH_W, (nch + 1) * NCH_W)
                    nc.scalar.activation(ynum_sb[:, nsl], pas[j][:], AFT.Copy)
                    nc.vector.tensor_copy(yden_sb[:, nsl], pbs[j][:])
                    nc.sync.dma_start(ynum_o[:, nsl], ynum_sb[:, nsl])
                    nc.sync.dma_start(yden_o[:, nsl], yden_sb[:, nsl])

    nc.compile()
    return nc


def _shard_inputs(inputs):
    """Build the 8 per-core input maps (host-side layout preparation)."""
    f32 = np.float32
    u = np.asarray(inputs["u_features"], dtype=f32)
    v = np.asarray(inputs["v_features"], dtype=f32)
    e = np.asarray(inputs["edge_features"], dtype=f32)
    M = np.asarray(inputs["node_edge_matrix"], dtype=f32)
    W_e = np.asarray(inputs["W_e"], dtype=f32)
    b_e = np.asarray(inputs["b_e"], dtype=f32)
    W = np.asarray(inputs["W"], dtype=f32)
    b = np.asarray(inputs["b"], dtype=f32)
    W_a = np.asarray(inputs["W_a"], dtype=f32)
    b_a = np.asarray(inputs["b_a"], dtype=f32)

    shared = {
        "we_u": np.ascontiguousarray(W_e[0:128]),
        "we_e": np.ascontiguousarray(W_e[128:192]),
        "we_v": np.ascontiguousarray(W_e[192:320]),
        "w_u": np.ascontiguousarray(W[0:128]),
        "w_ne": np.ascontiguousarray(W[128:192]),
        "w_v": np.ascontiguousarray(W[192:320]),
        "b_e": np.ascontiguousarray(b_e.reshape(E_DIM, 1)),
        "b": np.ascontiguousarray(b.reshape(C_DIM, 1)),
        "w_a": np.ascontiguousarray(np.repeat(W_a.reshape(E_DIM, 1), 2, axis=1)),
        "b_a": np.full((128, 1), float(b_a.reshape(-1)[0]), dtype=f32),
        "ident": np.eye(128, dtype=f32),
    }

    in_maps = []
    for k in range(N_CORES):
        sl = slice(k * E_LOC, (k + 1) * E_LOC)
        # M[:, sl].T -> [E_LOC, N]; pack as [NCH][128 part][EB*512] so each
        # node-chunk is one fully-contiguous 4 MiB DMA:
        #   mt[nch, p, eb*512 + j] = M[nch*512 + j, sl][eb*128 + p]
        mt = (
            M[:, sl]
            .T.reshape(EB, 128, NCH, NCH_W)
            .transpose(2, 1, 0, 3)
            .reshape(NCH, 128, EB * NCH_W)
        )
        in_map = {
            "uT": np.ascontiguousarray(u[sl].T),
            "vT": np.ascontiguousarray(v[sl].T),
            "eT": np.ascontiguousarray(e[sl].T),
            "mt": np.ascontiguousarray(mt),
        }
        in_map.update(shared)
        in_maps.append(in_map)
    return in_maps


def _postprocess(results, want_ne_fallback_inputs=None):
    ynum = np.zeros((C_DIM, N_NODES), np.float64)
    yden = np.zeros((1, N_NODES), np.float64)
    ne_slabs = []
    for res in results:
        ynum += res["ynum_out"]
        yden += res["yden_out"]
        ne_slabs.append(np.asarray(res["neT_out"]).T)
    new_edge = np.concatenate(ne_slabs, axis=0).astype(np.float32)
    ratio = (ynum / yden).T.astype(np.float32)  # [N, C]
    context = np.where(ratio > 0, ratio, np.expm1(ratio)).astype(np.float32)
    return context, new_edge


def kernel(**inputs):
    from concourse.bass_utils import run_bass_kernel_spmd

    if "nc" not in _CACHE:
        _CACHE["nc"] = _build_program()
    nc = _CACHE["nc"]

    in_maps = _shard_inputs(inputs)
    out = run_bass_kernel_spmd(nc, in_maps, core_ids=list(range(N_CORES)))
    context, new_edge = _postprocess(out.results)
    return context, new_edge


# revision 14
# speedup vs baseline: 1.5678x; 1.0544x over previous
"""Trainium2 Bass kernel for nn_ConcatMesPassing (GNN message passing).

Reference computation:
    ne   = leaky_relu([u, e, v] @ W_e + b_e)                       # [E, 64]
    nbr  = leaky_relu([u, ne, v] @ W + b)                          # [E, 128]
    a    = leaky_relu(ne @ W_a + b_a)                              # [E]
    w    = softmax(node_edge_matrix * a + node_edge_mask, axis=1)  # [N, E]
    ctx  = elu(w @ nbr)                                            # [N, 128]
    returns (ctx, ne)

Key structural facts used here:
  * node_edge_matrix is binary and node_edge_mask == where(M>0, 0, -1e9),
    so the masked softmax row i is exactly  M_ij*exp(a_j) / sum_j M_ij*exp(a_j)
    (a is O(1) so exp needs no row-max subtraction).  The 256 MiB mask tensor
    never needs to be read.
  * ctx = elu( (M @ (s*nbr)) / (M @ s) )  with s = exp(a).

Sharding: edges are sharded 8 ways (2048 edges/core).  Each core runs the
edge MLPs for its slab and computes partial node aggregates
    ynum_k = (s*nbr)_k^T @ M[:, slab_k]^T   ([128, 4096], transposed)
    yden_k = s_k^T @ M[:, slab_k]^T         ([1, 4096])
over its 2048 columns of M.  The host sums the 8 partials, divides and
applies elu.  M column-slabs are pre-transposed/packed on the host so the
device streams them as fully-contiguous DMA loads and feeds the PE array
directly (contraction dim = edges on partitions).
"""

import os
import sys

import numpy as np

try:
    import ml_dtypes
except ImportError:  # bf16 numpy dtype
    ml_dtypes = None

for _p in ("/opt/trn_rl_repo", "/root/.axon_site/_ro/trn_rl_repo"):
    if os.path.isdir(_p) and _p not in sys.path:
        sys.path.append(_p)

N_NODES = 4096
E_TOTAL = 16384
N_DIM = 128
E_DIM = 64
C_DIM = 128
N_CORES = 8
E_LOC = E_TOTAL // N_CORES          # 2048 edges per core
EB = E_LOC // 128                   # 16 edge blocks of 128
NCH = 8                             # node chunks per core
NCH_W = N_NODES // NCH              # 512 nodes per chunk

_CACHE = {}


def _build_program(use_lrelu=True):
    import concourse.bass as bass
    import concourse.mybir as mybir
    import concourse.tile as tile
    from concourse import bacc

    f32 = mybir.dt.float32
    f32r = mybir.dt.float32r
    bf16 = mybir.dt.bfloat16
    AFT = mybir.ActivationFunctionType
    LRELU = AFT.Lrelu if use_lrelu else AFT.Relu  # Relu only for CoreSim checks

    nc = bacc.Bacc(None, target_bir_lowering=False, debug=False)

    # ---- DRAM I/O (per-core shards supplied by the host) ----
    uT_d = nc.dram_tensor("uT", [N_DIM, E_LOC], f32r, kind="ExternalInput").ap()
    vT_d = nc.dram_tensor("vT", [N_DIM, E_LOC], f32r, kind="ExternalInput").ap()
    eT_d = nc.dram_tensor("eT", [E_DIM, E_LOC], f32r, kind="ExternalInput").ap()
    mt_d = nc.dram_tensor("mt", [NCH, 128, EB * NCH_W], bf16, kind="ExternalInput").ap()
    we_u_d = nc.dram_tensor("we_u", [128, E_DIM], f32r, kind="ExternalInput").ap()
    we_e_d = nc.dram_tensor("we_e", [64, E_DIM], f32r, kind="ExternalInput").ap()
    we_v_d = nc.dram_tensor("we_v", [128, E_DIM], f32r, kind="ExternalInput").ap()
    w_u_d = nc.dram_tensor("w_u", [128, C_DIM], f32r, kind="ExternalInput").ap()
    w_ne_d = nc.dram_tensor("w_ne", [64, C_DIM], f32r, kind="ExternalInput").ap()
    w_v_d = nc.dram_tensor("w_v", [128, C_DIM], f32r, kind="ExternalInput").ap()
    b_e_d = nc.dram_tensor("b_e", [E_DIM, 1], f32, kind="ExternalInput").ap()
    b_d = nc.dram_tensor("b", [C_DIM, 1], f32, kind="ExternalInput").ap()
    w_a_d = nc.dram_tensor("w_a", [E_DIM, 2], f32r, kind="ExternalInput").ap()  # fp32r needs even N
    b_a_d = nc.dram_tensor("b_a", [128, 1], f32, kind="ExternalInput").ap()
    id_d = nc.dram_tensor("ident", [128, 128], f32r, kind="ExternalInput").ap()

    neT_o = nc.dram_tensor("neT_out", [E_DIM, E_LOC], f32r, kind="ExternalOutput").ap()
    ynum_o = nc.dram_tensor("ynum_out", [C_DIM, N_NODES], f32, kind="ExternalOutput").ap()
    yden_o = nc.dram_tensor("yden_out", [1, N_NODES], f32, kind="ExternalOutput").ap()

    def r(ap):
        return ap.bitcast(f32r)

    with tile.TileContext(nc) as tc:
        with (
            tc.tile_pool(name="const", bufs=1) as cpool,
            tc.tile_pool(name="feat", bufs=1) as fpool,
            tc.tile_pool(name="mtp", bufs=6) as mtpool,
            tc.tile_pool(name="outp", bufs=1) as opool,
            tc.tile_pool(name="pst", bufs=2, space="PSUM") as pst,
            tc.tile_pool(name="acc", bufs=3, space="PSUM") as acc,
        ):
            # constants
            we_u = cpool.tile([128, E_DIM], f32r)
            we_e = cpool.tile([64, E_DIM], f32r)
            we_v = cpool.tile([128, E_DIM], f32r)
            w_u = cpool.tile([128, C_DIM], f32r)
            w_ne = cpool.tile([64, C_DIM], f32r)
            w_v = cpool.tile([128, C_DIM], f32r)
            b_e = cpool.tile([E_DIM, 1], f32)
            b_c = cpool.tile([C_DIM, 1], f32)
            w_a = cpool.tile([E_DIM, 2], f32r)
            b_a = cpool.tile([128, 1], f32)
            ident = cpool.tile([128, 128], f32r)
            for sb, dr in [
                (we_u, we_u_d), (we_e, we_e_d), (we_v, we_v_d),
                (w_u, w_u_d), (w_ne, w_ne_d), (w_v, w_v_d),
                (b_e, b_e_d), (b_c, b_d), (w_a, w_a_d), (b_a, b_a_d),
                (ident, id_d),
            ]:
                nc.sync.dma_start(sb[:], dr)

            uT = fpool.tile([N_DIM, E_LOC], f32r)
            vT = fpool.tile([N_DIM, E_LOC], f32r)
            eT = fpool.tile([E_DIM, E_LOC], f32r)
            nc.sync.dma_start(uT[:], uT_d)
            nc.sync.dma_start(vT[:], vT_d)
            nc.sync.dma_start(eT[:], eT_d)

            neT = fpool.tile([E_DIM, E_LOC], f32r)
            nbrT = fpool.tile([C_DIM, E_LOC], f32r)
            sN = fpool.tile([128, 2 * EB], f32)   # exp(a), edge-major, col 2*eb per block
            gsc = fpool.tile([128, EB * C_DIM], bf16)  # s * nbr (bf16), edge-major
            sNr = fpool.tile([128, 2 * EB], bf16)  # bf16 copy for the den matmul
            ynum_sb = opool.tile([C_DIM, N_NODES], f32)
            yden_sb = opool.tile([1, N_NODES], f32)

            # ---- MLP1: neT = leaky(W_e^T [u;e;v]) ----
            for c4 in range(4):
                sl = slice(c4 * 512, (c4 + 1) * 512)
                ps1 = pst.tile([E_DIM, 512], f32, tag="ps", name=f"ps1_{c4}")
                nc.tensor.matmul(ps1[:], we_u[:], uT[:, sl], start=True, stop=False)
                nc.tensor.matmul(ps1[:], we_e[:], eT[:, sl], start=False, stop=False)
                nc.tensor.matmul(ps1[:], we_v[:], vT[:, sl], start=False, stop=True)
                nc.scalar.activation(neT[:, sl], ps1[:], LRELU, bias=b_e[:], alpha=0.01)
            nc.sync.dma_start(neT_o, neT[:])

            # ---- MLP2: nbrT = leaky(W^T [u;ne;v]) ----
            for c4 in range(4):
                sl = slice(c4 * 512, (c4 + 1) * 512)
                ps2 = pst.tile([C_DIM, 512], f32, tag="ps", name=f"ps2_{c4}")
                nc.tensor.matmul(ps2[:], w_u[:], uT[:, sl], start=True, stop=False)
                nc.tensor.matmul(ps2[:], w_ne[:], neT[:, sl], start=False, stop=False)
                nc.tensor.matmul(ps2[:], w_v[:], vT[:, sl], start=False, stop=True)
                nc.scalar.activation(nbrT[:, sl], ps2[:], LRELU, bias=b_c[:], alpha=0.01)

            # ---- attention scalars, edge-major: s = exp(leaky(ne @ W_a + b_a)) ----
            # All 16 per-block matmuls land in one [128, EB] psum tile so the
            # Lrelu and Exp each run once (ACT table reloads on every function
            # switch, ~1.3us each -- batching cut 32 reloads to 2).
            pss = pst.tile([128, 2 * EB], f32, tag="ps", name="pss")
            for eb in range(EB):
                esl = slice(eb * 128, (eb + 1) * 128)
                nc.tensor.matmul(pss[:, 2 * eb:2 * eb + 2], neT[:, esl], w_a[:],
                                 start=True, stop=True)
            lr = fpool.tile([128, 2 * EB], f32, name="lr")
            nc.scalar.activation(lr[:], pss[:], LRELU, bias=b_a[:], alpha=0.01)
            nc.scalar.activation(sN[:], lr[:], AFT.Exp)

            # ---- G = s * nbr in edge-major blocks (transpose nbrT via PE) ----
            for eb in range(EB):
                esl = slice(eb * 128, (eb + 1) * 128)
                pstr = pst.tile([128, C_DIM], f32r, tag="ps", name=f"pstr_{eb}")
                nc.tensor.transpose(pstr[:], nbrT[:, esl], ident[:])
                nc.scalar.activation(
                    gsc[:, eb * C_DIM:(eb + 1) * C_DIM], pstr[:], AFT.Copy,
                    scale=sN[:, 2 * eb:2 * eb + 1],
                )

            nc.vector.tensor_copy(sNr[:], sN[:])

            # ---- aggregation over M columns: ynum += G^T @ MT, yden += s^T @ MT ----
            # Node chunks are processed in pairs that share each gsc[eb]/sN[eb]
            # stationary load (halves LDWEIGHTS) and keep the PE dense.  M^T
            # streams as 2 MiB halves on the scalar-engine HWDGE ring so the
            # feature loads on the sync ring aren't stuck behind the prefetch
            # queue.  Outputs drain per-chunk to overlap stores with compute.
            HEB = EB // 2  # 8 edge blocks per half-chunk
            for pr in range(NCH // 2):
                pas = [acc.tile([C_DIM, NCH_W], f32, tag="pa", name=f"pa_{pr}_{j}")
                       for j in range(2)]
                pbs = [acc.tile([1, NCH_W], f32, tag="pb", name=f"pb_{pr}_{j}")
                       for j in range(2)]
                for h in range(2):
                    mts = []
                    for j in range(2):
                        nch = 2 * pr + j
                        mtc = mtpool.tile([128, HEB * NCH_W], bf16, tag="mtc",
                                          name=f"mtc_{nch}_{h}")
                        nc.scalar.dma_start(
                            mtc[:],
                            mt_d[nch][:, h * HEB * NCH_W:(h + 1) * HEB * NCH_W],
                        )
                        mts.append(mtc)
                    for k in range(HEB):
                        eb = h * HEB + k
                        st, sp = (eb == 0), (eb == EB - 1)
                        g = gsc[:, eb * C_DIM:(eb + 1) * C_DIM]
                        sv = sNr[:, 2 * eb:2 * eb + 1]
                        for j in range(2):
                            rhs = mts[j][:, k * NCH_W:(k + 1) * NCH_W]
                            nc.tensor.matmul(pas[j][:], g, rhs, start=st, stop=sp)
                        for j in range(2):
                            rhs = mts[j][:, k * NCH_W:(k + 1) * NCH_W]
                            nc.tensor.matmul(pbs[j][:], sv, rhs, start=st, stop=sp)
                for j in range(2):
                    nch = 2 * pr + j
                    nsl = slice(nch * NCH_W, (nch + 1) * NCH_W)
                    nc.scalar.activation(ynum_sb[:, nsl], pas[j][:], AFT.Copy)
                    nc.vector.tensor_copy(yden_sb[:, nsl], pbs[j][:])
                    nc.sync.dma_start(ynum_o[:, nsl], ynum_sb[:, nsl])
                    nc.sync.dma_start(yden_o[:, nsl], yden_sb[:, nsl])

    nc.compile()
    return nc


def _shard_inputs(inputs):
    """Build the 8 per-core input maps (host-side layout preparation)."""
    f32 = np.float32
    u = np.asarray(inputs["u_features"], dtype=f32)
    v = np.asarray(inputs["v_features"], dtype=f32)
    e = np.asarray(inputs["edge_features"], dtype=f32)
    M = np.asarray(inputs["node_edge_matrix"], dtype=f32)
    W_e = np.asarray(inputs["W_e"], dtype=f32)
    b_e = np.asarray(inputs["b_e"], dtype=f32)
    W = np.asarray(inputs["W"], dtype=f32)
    b = np.asarray(inputs["b"], dtype=f32)
    W_a = np.asarray(inputs["W_a"], dtype=f32)
    b_a = np.asarray(inputs["b_a"], dtype=f32)

    shared = {
        "we_u": np.ascontiguousarray(W_e[0:128]),
        "we_e": np.ascontiguousarray(W_e[128:192]),
        "we_v": np.ascontiguousarray(W_e[192:320]),
        "w_u": np.ascontiguousarray(W[0:128]),
        "w_ne": np.ascontiguousarray(W[128:192]),
        "w_v": np.ascontiguousarray(W[192:320]),
        "b_e": np.ascontiguousarray(b_e.reshape(E_DIM, 1)),
        "b": np.ascontiguousarray(b.reshape(C_DIM, 1)),
        "w_a": np.ascontiguousarray(np.repeat(W_a.reshape(E_DIM, 1), 2, axis=1)),
        "b_a": np.full((128, 1), float(b_a.reshape(-1)[0]), dtype=f32),
        "ident": np.eye(128, dtype=f32),
    }

    in_maps = []
    for k in range(N_CORES):
        sl = slice(k * E_LOC, (k + 1) * E_LOC)
        # M[:, sl].T -> [E_LOC, N]; pack as [NCH][128 part][EB*512] so each
        # node-chunk is one fully-contiguous 4 MiB DMA:
        #   mt[nch, p, eb*512 + j] = M[nch*512 + j, sl][eb*128 + p]
        mt = (
            M[:, sl]
            .T.reshape(EB, 128, NCH, NCH_W)
            .transpose(2, 1, 0, 3)
            .reshape(NCH, 128, EB * NCH_W)
            .astype(ml_dtypes.bfloat16)  # M is 0/1 -> exact in bf16, halves HBM
        )
        in_map = {
            "uT": np.ascontiguousarray(u[sl].T),
            "vT": np.ascontiguousarray(v[sl].T),
            "eT": np.ascontiguousarray(e[sl].T),
            "mt": np.ascontiguousarray(mt),
        }
        in_map.update(shared)
        in_maps.append(in_map)
    return in_maps


def _postprocess(results, want_ne_fallback_inputs=None):
    ynum = np.zeros((C_DIM, N_NODES), np.float64)
    yden = np.zeros((1, N_NODES), np.float64)
    ne_slabs = []
    for res in results:
        ynum += res["ynum_out"]
        yden += res["yden_out"]
        ne_slabs.append(np.asarray(res["neT_out"]).T)
    new_edge = np.concatenate(ne_slabs, axis=0).astype(np.float32)
    ratio = (ynum / yden).T.astype(np.float32)  # [N, C]
    context = np.where(ratio > 0, ratio, np.expm1(ratio)).astype(np.float32)
    return context, new_edge


def kernel(**inputs):
    from concourse.bass_utils import run_bass_kernel_spmd

    if "nc" not in _CACHE:
        _CACHE["nc"] = _build_program()
    nc = _CACHE["nc"]

    in_maps = _shard_inputs(inputs)
    out = run_bass_kernel_spmd(nc, in_maps, core_ids=list(range(N_CORES)))
    context, new_edge = _postprocess(out.results)
    return context, new_edge


# revision 17
# speedup vs baseline: 2.2344x; 1.4252x over previous
"""Trainium2 Bass kernel for nn_ConcatMesPassing (GNN message passing).

Reference computation:
    ne   = leaky_relu([u, e, v] @ W_e + b_e)                       # [E, 64]
    nbr  = leaky_relu([u, ne, v] @ W + b)                          # [E, 128]
    a    = leaky_relu(ne @ W_a + b_a)                              # [E]
    w    = softmax(node_edge_matrix * a + node_edge_mask, axis=1)  # [N, E]
    ctx  = elu(w @ nbr)                                            # [N, 128]
    returns (ctx, ne)

Key structural facts used here:
  * node_edge_matrix is binary and node_edge_mask == where(M>0, 0, -1e9),
    so the masked softmax row i is exactly  M_ij*exp(a_j) / sum_j M_ij*exp(a_j)
    (a is O(1) so exp needs no row-max subtraction).  The 256 MiB mask tensor
    never needs to be read.
  * ctx = elu( (M @ (s*nbr)) / (M @ s) )  with s = exp(a).

Sharding: edges are sharded 8 ways (2048 edges/core).  Each core runs the
edge MLPs for its slab and computes partial node aggregates
    ynum_k = (s*nbr)_k^T @ M[:, slab_k]^T   ([128, 4096], transposed)
    yden_k = s_k^T @ M[:, slab_k]^T         ([1, 4096])
over its 2048 columns of M.  The host sums the 8 partials, divides and
applies elu.  M column-slabs are pre-transposed/packed on the host so the
device streams them as fully-contiguous DMA loads and feeds the PE array
directly (contraction dim = edges on partitions).
"""

import os
import sys

import numpy as np

try:
    import ml_dtypes
except ImportError:  # bf16 numpy dtype
    ml_dtypes = None

for _p in ("/opt/trn_rl_repo", "/root/.axon_site/_ro/trn_rl_repo"):
    if os.path.isdir(_p) and _p not in sys.path:
        sys.path.append(_p)

N_NODES = 4096
E_TOTAL = 16384
N_DIM = 128
E_DIM = 64
C_DIM = 128
N_CORES = 8
E_LOC = E_TOTAL // N_CORES          # 2048 edges per core
EB = E_LOC // 128                   # 16 edge blocks of 128
NCH = 8                             # node chunks per core
NCH_W = N_NODES // NCH              # 512 nodes per chunk

_CACHE = {}


def _build_program(use_lrelu=True):
    import concourse.bass as bass
    import concourse.mybir as mybir
    import concourse.tile as tile
    from concourse import bacc

    f32 = mybir.dt.float32
    f32r = mybir.dt.float32r
    bf16 = mybir.dt.bfloat16
    AFT = mybir.ActivationFunctionType
    LRELU = AFT.Lrelu if use_lrelu else AFT.Relu  # Relu only for CoreSim checks

    nc = bacc.Bacc(None, target_bir_lowering=False, debug=False)

    # ---- DRAM I/O (per-core shards supplied by the host) ----
    # One packed constants tensor and one packed u|v tensor: each dma_start
    # costs ~0.6us of sequencer issue + wait, and 14 small loads were
    # serializing the head for ~20us before MLP1 could start.
    pk_d = nc.dram_tensor("pk", [128, 712], f32r, kind="ExternalInput").ap()
    uv_d = nc.dram_tensor("uv", [N_DIM, 2 * E_LOC], f32r, kind="ExternalInput").ap()
    eT_d = nc.dram_tensor("eT", [E_DIM, E_LOC], f32r, kind="ExternalInput").ap()
    mt_d = nc.dram_tensor("mt", [NCH, 128, EB * NCH_W], bf16, kind="ExternalInput").ap()

    neT_o = nc.dram_tensor("neT_out", [E_DIM, E_LOC], f32r, kind="ExternalOutput").ap()
    ynum_o = nc.dram_tensor("ynum_out", [C_DIM, N_NODES], f32, kind="ExternalOutput").ap()

    def r(ap):
        return ap.bitcast(f32r)

    with tile.TileContext(nc) as tc:
        with (
            tc.tile_pool(name="const", bufs=1) as cpool,
            tc.tile_pool(name="feat", bufs=1) as fpool,
            tc.tile_pool(name="mtp", bufs=6) as mtpool,
            tc.tile_pool(name="outp", bufs=1) as opool,
            tc.tile_pool(name="pst", bufs=2, space="PSUM") as pst,
            tc.tile_pool(name="acc", bufs=3, space="PSUM") as acc,
        ):
            # constants (single packed load, sliced below)
            pk = cpool.tile([128, 712], f32r)
            nc.sync.dma_start(pk[:], pk_d)
            we_u, we_e, we_v = pk[:, 0:64], pk[0:64, 64:128], pk[:, 128:192]
            w_u, w_ne, w_v = pk[:, 192:320], pk[0:64, 320:448], pk[:, 448:576]
            ident = pk[:, 576:704]
            b_e = pk[0:64, 704:705].bitcast(f32)
            b_c = pk[:, 705:706].bitcast(f32)
            w_a = pk[0:64, 706:708]
            b_a = pk[:, 708:709].bitcast(f32)

            uv = fpool.tile([N_DIM, 2 * E_LOC], f32r)
            eT = fpool.tile([E_DIM, E_LOC], f32r)
            nc.sync.dma_start(uv[:], uv_d)
            nc.sync.dma_start(eT[:], eT_d)
            uT, vT = uv[:, 0:E_LOC], uv[:, E_LOC:2 * E_LOC]

            neT = fpool.tile([E_DIM, E_LOC], f32r)
            nbrT = fpool.tile([C_DIM, E_LOC], f32r)
            sN = fpool.tile([128, 2 * EB], f32)   # exp(a), edge-major, col 2*eb per block
            gsc = fpool.tile([128, EB * C_DIM], bf16)  # s * nbr (bf16), edge-major
            ynum_sb = opool.tile([C_DIM, N_NODES], f32)

            # ---- MLP1: neT = leaky(W_e^T [u;e;v]) ----
            for c4 in range(4):
                sl = slice(c4 * 512, (c4 + 1) * 512)
                ps1 = pst.tile([E_DIM, 512], f32, tag="ps", name=f"ps1_{c4}")
                nc.tensor.matmul(ps1[:], we_u, uT[:, sl], start=True, stop=False)
                nc.tensor.matmul(ps1[:], we_e, eT[:, sl], start=False, stop=False)
                nc.tensor.matmul(ps1[:], we_v, vT[:, sl], start=False, stop=True)
                nc.scalar.activation(neT[:, sl], ps1[:], LRELU, bias=b_e, alpha=0.01)
            nc.sync.dma_start(neT_o, neT[:])

            # ---- MLP2: nbrT = leaky(W^T [u;ne;v]) ----
            for c4 in range(4):
                sl = slice(c4 * 512, (c4 + 1) * 512)
                ps2 = pst.tile([C_DIM, 512], f32, tag="ps", name=f"ps2_{c4}")
                nc.tensor.matmul(ps2[:], w_u, uT[:, sl], start=True, stop=False)
                nc.tensor.matmul(ps2[:], w_ne, neT[:, sl], start=False, stop=False)
                nc.tensor.matmul(ps2[:], w_v, vT[:, sl], start=False, stop=True)
                nc.scalar.activation(nbrT[:, sl], ps2[:], LRELU, bias=b_c, alpha=0.01)

            # ---- attention scalars, edge-major: s = exp(leaky(ne @ W_a + b_a)) ----
            # All 16 per-block matmuls land in one [128, EB] psum tile so the
            # Lrelu and Exp each run once (ACT table reloads on every function
            # switch, ~1.3us each -- batching cut 32 reloads to 2).
            pss = pst.tile([128, 2 * EB], f32, tag="ps", name="pss")
            for eb in range(EB):
                esl = slice(eb * 128, (eb + 1) * 128)
                nc.tensor.matmul(pss[:, 2 * eb:2 * eb + 2], neT[:, esl], w_a,
                                 start=True, stop=True)
            lr = fpool.tile([128, 2 * EB], f32, name="lr")
            nc.scalar.activation(lr[:], pss[:], LRELU, bias=b_a, alpha=0.01)
            nc.scalar.activation(sN[:], lr[:], AFT.Exp)

            # ---- G = s * nbr in edge-major blocks (transpose nbrT via PE) ----
            for eb in range(EB):
                esl = slice(eb * 128, (eb + 1) * 128)
                pstr = pst.tile([128, C_DIM], f32r, tag="ps", name=f"pstr_{eb}")
                nc.tensor.transpose(pstr[:], nbrT[:, esl], ident)
                nc.scalar.activation(
                    gsc[:, eb * C_DIM:(eb + 1) * C_DIM], pstr[:], AFT.Copy,
                    scale=sN[:, 2 * eb:2 * eb + 1],
                )

            # ---- aggregation over M columns: ynum += G^T @ MT, yden += s^T @ MT ----
            # Node chunks are processed in pairs that share each gsc[eb]/sN[eb]
            # stationary load (halves LDWEIGHTS) and keep the PE dense.  M^T
            # streams as 2 MiB halves on the scalar-engine HWDGE ring so the
            # feature loads on the sync ring aren't stuck behind the prefetch
            # queue.  Outputs drain per-chunk to overlap stores with compute.
            HEB = EB // 2  # 8 edge blocks per half-chunk
            for pr in range(NCH // 2):
                pas = [acc.tile([C_DIM, NCH_W], f32, tag="pa", name=f"pa_{pr}_{j}")
                       for j in range(2)]
                for h in range(2):
                    mts = []
                    for j in range(2):
                        nch = 2 * pr + j
                        mtc = mtpool.tile([128, HEB * NCH_W], bf16, tag="mtc",
                                          name=f"mtc_{nch}_{h}")
                        nc.scalar.dma_start(
                            mtc[:],
                            mt_d[nch][:, h * HEB * NCH_W:(h + 1) * HEB * NCH_W],
                        )
                        mts.append(mtc)
                    for k in range(HEB):
                        eb = h * HEB + k
                        st, sp = (eb == 0), (eb == EB - 1)
                        g = gsc[:, eb * C_DIM:(eb + 1) * C_DIM]
                        for j in range(2):
                            rhs = mts[j][:, k * NCH_W:(k + 1) * NCH_W]
                            nc.tensor.matmul(pas[j][:], g, rhs, start=st, stop=sp)
                for j in range(2):
                    nch = 2 * pr + j
                    nsl = slice(nch * NCH_W, (nch + 1) * NCH_W)
                    nc.scalar.activation(ynum_sb[:, nsl], pas[j][:], AFT.Copy)
                    nc.sync.dma_start(ynum_o[:, nsl], ynum_sb[:, nsl])

    nc.compile()
    return nc


def _shard_inputs(inputs):
    """Build the 8 per-core input maps (host-side layout preparation)."""
    f32 = np.float32
    u = np.asarray(inputs["u_features"], dtype=f32)
    v = np.asarray(inputs["v_features"], dtype=f32)
    e = np.asarray(inputs["edge_features"], dtype=f32)
    M = np.asarray(inputs["node_edge_matrix"], dtype=f32)
    W_e = np.asarray(inputs["W_e"], dtype=f32)
    b_e = np.asarray(inputs["b_e"], dtype=f32)
    W = np.asarray(inputs["W"], dtype=f32)
    b = np.asarray(inputs["b"], dtype=f32)
    W_a = np.asarray(inputs["W_a"], dtype=f32)
    b_a = np.asarray(inputs["b_a"], dtype=f32)

    pk = np.zeros((128, 712), np.float32)
    pk[:, 0:64] = W_e[0:128]
    pk[0:64, 64:128] = W_e[128:192]
    pk[:, 128:192] = W_e[192:320]
    pk[:, 192:320] = W[0:128]
    pk[0:64, 320:448] = W[128:192]
    pk[:, 448:576] = W[192:320]
    pk[:, 576:704] = np.eye(128, dtype=f32)
    pk[0:64, 704] = b_e
    pk[:, 705] = b
    pk[0:64, 706] = W_a[:, 0]
    pk[0:64, 707] = W_a[:, 0]
    pk[:, 708] = float(b_a.reshape(-1)[0])
    shared = {"pk": pk}

    in_maps = []
    for k in range(N_CORES):
        sl = slice(k * E_LOC, (k + 1) * E_LOC)
        # M[:, sl].T -> [E_LOC, N]; pack as [NCH][128 part][EB*512] so each
        # node-chunk is one fully-contiguous 4 MiB DMA:
        #   mt[nch, p, eb*512 + j] = M[nch*512 + j, sl][eb*128 + p]
        mt = (
            M[:, sl]
            .T.reshape(EB, 128, NCH, NCH_W)
            .transpose(2, 1, 0, 3)
            .reshape(NCH, 128, EB * NCH_W)
            .astype(ml_dtypes.bfloat16)  # M is 0/1 -> exact in bf16, halves HBM
        )
        in_map = {
            "uv": np.ascontiguousarray(np.concatenate([u[sl].T, v[sl].T], axis=1)),
            "eT": np.ascontiguousarray(e[sl].T),
            "mt": np.ascontiguousarray(mt),
        }
        in_map.update(shared)
        in_maps.append(in_map)
    return in_maps


def _postprocess(results, W_a, b_a):
    ynum = np.zeros((C_DIM, N_NODES), np.float64)
    ne_slabs = []
    for res in results:
        ynum += res["ynum_out"]
        ne_slabs.append(np.asarray(res["neT_out"]).T)
    new_edge = np.concatenate(ne_slabs, axis=0).astype(np.float32)
    # softmax denominator from the device-computed edge features: one matvec
    # over M on the host (the device already did the O(N*E*C) numerator).
    z = new_edge @ np.asarray(W_a, np.float32) + np.asarray(b_a, np.float32)
    a = np.where(z > 0, z, np.float32(0.01) * z).reshape(-1)
    s = np.exp(a, dtype=np.float32)
    return ynum, new_edge, s


def kernel(**inputs):
    from concourse.bass_utils import run_bass_kernel_spmd

    if "nc" not in _CACHE:
        _CACHE["nc"] = _build_program()
    nc = _CACHE["nc"]

    in_maps = _shard_inputs(inputs)
    out = run_bass_kernel_spmd(nc, in_maps, core_ids=list(range(N_CORES)))
    ynum, new_edge, s = _postprocess(out.results, inputs["W_a"], inputs["b_a"])
    M = np.asarray(inputs["node_edge_matrix"], dtype=np.float32)
    yden = M @ s  # [N]
    ratio = (ynum / yden[None, :]).T.astype(np.float32)  # [N, C]
    context = np.where(ratio > 0, ratio, np.expm1(ratio)).astype(np.float32)
    return context, new_edge
